# revision 21
# baseline (speedup 1.0000x reference)
"""Trainium2 Bass kernel for a dense transformer block (B=4, T=2048, C=1024, H=16).

Sharding: data-parallel over tokens. Core i owns batch b=i//2, token-half i%2
(1024 tokens). Each core redundantly computes LN1/K/V for its batch's full 2048
tokens so there are no collectives.

v3: restructured from the v2 pipeline for engine balance:
- Phase A (LN1+K/V/Q): x is read from HBM once; f32->bf16 casts moved to the
  (otherwise idle) gpsimd engine; LN applies run on the kept bf16 copy in 2x
  DVE mode (mu/s cast to bf16); Q is emitted inside its own chunk so it
  pipelines with the LN chain.
- Attention: scores land in a 4-bank PSUM tile so exp runs at N=2048 per
  ACTIVATE (amortizes the 352-cycle ACT overhead). Scores matmuls contract
  K=64 directly (single q_all tile, no zero-padded half-tiles, half the Q
  epilogues, no qz memsets). Softmax normalize uses reciprocal_approx_fast
  (~5x cheaper than the iterative divide; denominators are >=0.1 so the
  approx is safe).
- MLP1(qc0) interleaves into qc1's head loop at m4 granularity (8-matmul
  units, ~1.7us) so the exp stream is never starved by long PE detours.
- relu on DVE everywhere -> ACT stays on the Ln/Exp table set (no reloads).
- x2 (= x + sa) is kept in SBUF as bf16 (no DRAM round-trip for LN2/MLP2
  residuals; the ~0.4% bf16 residual quantization is well inside the 2e-2
  gate).
- v_aug: the 1/32 denominator in column 0 (the custom-DVE reciprocal
  only reads partition 0) and v feats in columns 64-127 (>32-partition
  PSUM accesses must start at partition 0 or 64).

Scaling (unchanged from v2): fp8 weights stored x32; k/q carry x32 (folded
into the exp scale C^-0.5/1024); v descaled by 1/32 in its epilogue; the
attention denominator rides AV as a (1/32)-column of V so o comes out x32,
matching the x32 proj weights; the proj epilogue multiplies by 2^-10.

PSUM: sc4 [128,4,512] (scores / 4 rotating K/V/Q accums / MLP2 accums 0-3)
+ avp [128,512] x2 (AV / proj / MLP2 accums) + st [128,2,512] (LN stats,
MLP1-interleave accums, MLP2 accums) = 8 banks.
"""

import sys

if "/opt/trn_rl_repo" not in sys.path:
    sys.path.insert(0, "/opt/trn_rl_repo")

import numpy as np
import ml_dtypes

B, T, C, H, HD = 4, 2048, 1024, 16, 64
FF = 4 * C
TO = T // 2          # tokens owned per core
NCC = C // 128       # 8
NFC = FF // 128      # 32
EPS = 1e-5
SCALE = C ** -0.5    # 1/32
ESCALE = SCALE / 1024.0   # exp scale with k,q carried x32
BF16 = ml_dtypes.bfloat16
F8NP = ml_dtypes.float8_e4m3

_BUILT = None
_DEBUG = False   # when True, _build adds intermediate-dump outputs


def _emit(nc, tc, aps, has_bv, has_bp):
    from concourse import mybir
    from concourse.bass import ts
    F32 = mybir.dt.float32
    BF = mybir.dt.bfloat16
    F8 = mybir.dt.float8e4
    AF = mybir.ActivationFunctionType
    PM = mybir.MatmulPerfMode
    ADD = mybir.AluOpType.add
    MULT = mybir.AluOpType.mult
    MAX = mybir.AluOpType.max
    from contextlib import ExitStack

    xT, wq, wk, wv, wproj, w1, w2, outT = (
        aps["xT"], aps["wq"], aps["wk"], aps["wv"], aps["wproj"], aps["w1"],
        aps["w2"], aps["outT"])

    ctx = ExitStack()
    with ctx:
        const = ctx.enter_context(tc.tile_pool(name="const", bufs=1))
        misc = ctx.enter_context(tc.tile_pool(name="misc", bufs=2))
        psum = ctx.enter_context(tc.tile_pool(name="psum", bufs=1, space="PSUM"))

        def ps_sc4():
            return psum.tile([128, 4, 512], F32, name="ps_sc4", tag="sc4", bufs=1)

        def ps_st():
            return psum.tile([128, 2, 512], F32, name="ps_st", tag="st", bufs=1)

        def ps_av():
            return psum.tile([128, 512], F32, name="ps_av", tag="avp", bufs=2)

        def rot6_gen():
            """6 rotating [128,512] accumulators (sc4's 4 banks + 2 avp) for
            phases where attention PSUM is free."""
            while True:
                t = ps_sc4()
                for bb in range(4):
                    yield t[:, bb, :]
                yield ps_av()
                yield ps_av()

        def rot2_gen():
            """2 rotating accumulators clear of sc4/avp (for MLP1 interleaved
            into the attention head loop)."""
            while True:
                t = ps_st()
                yield t[:, 0, :]
                yield t[:, 1, :]

        # fresh generator per phase: a generator suspended mid-cycle holds a
        # live psum tile; resuming it in a later phase would make that stale
        # tile's release depend on later-phase work (deadlock)
        _rot6 = rot6_gen()
        _rot2 = rot2_gen()

        # constants; ones_sc/eps_sb are built via gpsimd so its ucode IRAM
        # loads overlap the initial DMAs instead of stalling the first real
        # cast/broadcast (~6us each)
        ones_f = const.tile([128, 128], F32, name="ones_f")
        nc.vector.memset(ones_f, 1.0 / C)
        ones_sc = const.tile([128, 128], BF, name="ones_sc")
        nc.gpsimd.tensor_copy(out=ones_sc, in_=ones_f)
        eps1 = const.tile([1, 1], F32, name="eps1")
        nc.vector.memset(eps1, EPS)
        eps_sb = const.tile([128, 1], F32, name="eps_sb")
        nc.gpsimd.partition_broadcast(eps_sb, eps1)

        bq_sb = const.tile([128, 8], F32, name="bq_sb")
        bk_sb = const.tile([128, 8], F32, name="bk_sb")
        bp_sb = const.tile([128, 8], F32, name="bp_sb")
        b2_sb = const.tile([128, 8], F32, name="b2_sb")
        b1_sb = const.tile([128, 32], F32, name="b1_sb")
        nc.sync.dma_start(out=bq_sb, in_=aps["bq"])
        nc.sync.dma_start(out=bk_sb, in_=aps["bk"])
        nc.sync.dma_start(out=bp_sb, in_=aps["bp"])
        nc.sync.dma_start(out=b2_sb, in_=aps["b2c"])
        nc.sync.dma_start(out=b1_sb, in_=aps["b1c"])
        if has_bv:
            bv_sb = const.tile([1, 1024], F32, name="bv_sb")
            nc.sync.dma_start(out=bv_sb, in_=aps["bvrow"])
            bvb = const.tile([128, 1024], F32, name="bvb")
            nc.gpsimd.partition_broadcast(bvb, bv_sb)

        def stats_finish(st, bf16_out):
            """st: psum [:,0]=mean, [:,1]=E[x^2] (rows replicated).
            1/sd via exp(-0.5*ln(var+eps)) so ACT stays on the exp table."""
            mu0 = misc.tile([128, 512], BF if bf16_out else F32,
                            name="mu0", tag="mstat", bufs=4)
            nc.vector.tensor_copy(out=mu0, in_=st[:, 0, :])
            musq = misc.tile([128, 512], F32, name="musq", tag="stat", bufs=2)
            nc.vector.tensor_mul(out=musq, in0=mu0, in1=mu0)
            var = misc.tile([128, 512], F32, name="var", tag="stat", bufs=2)
            nc.vector.tensor_sub(out=var, in0=st[:, 1, :], in1=musq)
            lnv = misc.tile([128, 512], F32, name="lnv", tag="stat", bufs=2)
            nc.scalar.activation(out=lnv, in_=var, func=AF.Ln, bias=eps_sb)
            s0 = misc.tile([128, 512], F32, name="s0", tag="mstat", bufs=4)
            nc.scalar.activation(out=s0, in_=lnv, func=AF.Exp, scale=-0.5)
            if bf16_out:
                sb = misc.tile([128, 512], BF, name="sbf", tag="msbf", bufs=2)
                nc.gpsimd.tensor_copy(out=sb, in_=s0)
                return mu0, sb
            return mu0, s0

        ln2_stats = {}
        h2_tiles = {}
        z_tiles = {}
        x2_tiles = {}
        _w1_tiles = {}

        with tc.tile_pool(name="attn", bufs=1) as attn:
            k_sb = attn.tile([128, 8, T], BF, name="k_sb")
            q_all = attn.tile([128, 8, TO], BF, name="q_all")
            o_sb = attn.tile([128, 8, TO], F8, name="o_sb")
            v_aug = attn.tile([128, 16, 16, 128], F8, name="v_aug")
            # denominator col 0 = 1/32 (partition 0 of the AV psum: the
            # custom-DVE reciprocal reads its input at partition offset 0
            # regardless of the AP, so the denominator must live there)
            nc.vector.memset(v_aug[:, :, :, 0:1], 1.0 / 32.0)

            # ---- phase A: LN1 chunks + K/V (+Q for own chunks) ----
            with tc.tile_pool(name="lnp", bufs=2) as lnp, \
                 tc.tile_pool(name="wkv", bufs=4) as wkv:
                wtk = wtv = wtq = None
                for tcg in range(4):
                    xbf = lnp.tile([128, 8, 512], BF, name="xbf", tag="xbf",
                                   bufs=2)
                    st = ps_st()
                    for c in range(NCC):
                        xs = lnp.tile([128, 512], F32, name="xs", tag="xs",
                                      bufs=4)
                        nc.sync.dma_start(out=xs, in_=xT[c, :, ts(tcg, 512)])
                        nc.gpsimd.tensor_copy(out=xbf[:, c, :], in_=xs)
                        xsq = lnp.tile([128, 512], BF, name="xsq", tag="xsq",
                                       bufs=3)
                        nc.vector.tensor_mul(out=xsq, in0=xbf[:, c, :],
                                             in1=xbf[:, c, :])
                        nc.tensor.matmul(st[:, 0, :], ones_sc, xbf[:, c, :],
                                         start=(c == 0), stop=(c == NCC - 1),
                                         skip_group_check=True)
                        nc.tensor.matmul(st[:, 1, :], ones_sc, xsq,
                                         start=(c == 0), stop=(c == NCC - 1),
                                         skip_group_check=True)
                    if tcg == 0:
                        # weights emitted after chunk-0 x loads so x wins the
                        # DMA queue race and the LN pipeline starts early
                        wtk = [wkv.tile([128, 2, 1024], F8, name="wtk",
                                        tag="wkt") for _ in range(4)]
                        wtv = [wkv.tile([128, 2, 1024], F8, name="wtv",
                                        tag="wvt") for _ in range(4)]
                        wtq = [wkv.tile([128, 2, 1024], F8, name="wtq",
                                        tag="wqt") for _ in range(4)]
                        for p in range(4):
                            nc.sync.dma_start(out=wtk[p], in_=wk[p])
                            nc.sync.dma_start(out=wtv[p], in_=wv[p])
                        for p in range(4):
                            nc.sync.dma_start(out=wtq[p], in_=wq[p])
                    mu_bf, s_bf = stats_finish(st, bf16_out=True)
                    h_t = lnp.tile([128, 8, 512], F8, name="h_t", tag="h",
                                   bufs=2)
                    for c in range(NCC):
                        d = lnp.tile([128, 512], BF, name="d", tag="dt",
                                     bufs=3)
                        nc.vector.tensor_sub(out=d, in0=xbf[:, c, :],
                                             in1=mu_bf)
                        nc.vector.tensor_mul(out=h_t[:, c, :], in0=d,
                                             in1=s_bf)
                    # K: out k' = 32k [128 feats, 512 tok]
                    for m in range(NCC):
                        kp = next(_rot6)
                        for p in range(4):
                            nc.tensor.matmul(kp, wtk[p][:, :, ts(m, 128)],
                                             h_t[:, 2 * p:2 * p + 2, :],
                                             start=(p == 0), stop=(p == 3),
                                             perf_mode=PM.DoubleRow,
                                             skip_group_check=True)
                        nc.scalar.activation(
                            out=k_sb[:, m, ts(tcg, 512)], in_=kp,
                            func=AF.Identity, bias=bk_sb[:, m:m + 1])
                    # V: out [128 tok, 512 feats]; epilogue x(1/32) -> fp8
                    for tt in range(4):
                        for nch in range(2):
                            vp = next(_rot6)
                            for p in range(4):
                                nc.tensor.matmul(
                                    vp, h_t[:, 2 * p:2 * p + 2,
                                            ts(tt, 128)],
                                    wtv[p][:, :, ts(nch, 512)],
                                    start=(p == 0), stop=(p == 3),
                                    perf_mode=PM.DoubleRow,
                                    skip_group_check=True)
                            dst = v_aug[:, tcg * 4 + tt,
                                        nch * 8:(nch + 1) * 8, 64:128]
                            src = vp.rearrange("p (h d) -> p h d", h=8)
                            if has_bv:
                                vt = misc.tile([128, 8, 64], F32,
                                               name="vt", tag="vt", bufs=2)
                                bslice = bvb[:, ts(nch, 512)].rearrange(
                                    "p (h d) -> p h d", h=8)
                                nc.vector.tensor_add(out=vt, in0=src,
                                                     in1=bslice)
                                nc.scalar.mul(dst, vt, 1.0 / 32.0)
                            else:
                                nc.scalar.mul(dst, src, 1.0 / 32.0)
                    # Q projection for own chunks: q' = 32q, both head
                    # halves in one epilogue (full 128 partitions)
                    if tcg < 2:
                        for m in range(NCC):
                            qp = next(_rot6)
                            for p in range(4):
                                nc.tensor.matmul(
                                    qp, wtq[p][:, :, ts(m, 128)],
                                    h_t[:, 2 * p:2 * p + 2, :],
                                    start=(p == 0), stop=(p == 3),
                                    perf_mode=PM.DoubleRow,
                                    skip_group_check=True)
                            nc.scalar.activation(
                                out=q_all[:, m, ts(tcg, 512)], in_=qp,
                                func=AF.Identity, bias=bq_sb[:, m:m + 1])

            # ---- attention + proj per query chunk; MLP1(qc0) rides
            #      inside qc1's head loop at m4 granularity ----
            # side="right": these outlive the attn pool (non-LIFO vs the
            # left-side stack), so they allocate from the other heap end
            mlpp = ctx.enter_context(
                tc.tile_pool(name="mlp", bufs=1, side="right"))
            w1p = ctx.enter_context(
                tc.tile_pool(name="w1p", bufs=2, side="right"))

            def ln2_apply(qc):
                mu2, s2 = ln2_stats[qc]
                h2 = mlpp.tile([128, 8, 512], BF, name="h2", tag="h2", bufs=1)
                x2 = x2_tiles[qc]
                for c in range(NCC):
                    d2 = misc.tile([128, 512], F32, name="d2", tag="d2",
                                   bufs=2)
                    nc.vector.tensor_sub(out=d2, in0=x2[:, c, :], in1=mu2)
                    nc.vector.tensor_mul(out=h2[:, c, :], in0=d2, in1=s2)
                h2_tiles[qc] = h2
                z_tiles[qc] = mlpp.tile([128, 32, 512], BF, name="z_sb",
                                        tag="z", bufs=1)

            def mlp1_unit(qc, u, rot):
                """One m4 unit (8 matmuls + relu) of MLP1 for query chunk qc.
                u in 0..31; mg = u//4, m4 = u%4."""
                mg, m4 = u // 4, u % 4
                h2 = h2_tiles[qc]
                z_sb = z_tiles[qc]
                if m4 == 0:
                    w1t = w1p.tile([128, 8, 512], BF, name="w1t", tag="w1s")
                    nc.sync.dma_start(out=w1t, in_=w1[mg])
                    _w1_tiles[(qc, mg)] = w1t
                w1t = _w1_tiles[(qc, mg)]
                acc = next(rot)
                for k in range(NCC):
                    nc.tensor.matmul(acc, w1t[:, k, ts(m4, 128)],
                                     h2[:, k, :], start=(k == 0),
                                     stop=(k == NCC - 1),
                                     skip_group_check=True)
                m = mg * 4 + m4
                nc.vector.tensor_scalar(
                    out=z_sb[:, m, :], in0=acc,
                    scalar1=b1_sb[:, m:m + 1], scalar2=0.0,
                    op0=ADD, op1=MAX)

            with tc.tile_pool(name="wpp", bufs=4) as wpp, \
                 tc.tile_pool(name="expp", bufs=2) as expp, \
                 tc.tile_pool(name="stage", bufs=3) as stage:
                wtp = [wpp.tile([128, 2, 1024], F8, name="wtp", tag="wp")
                       for _ in range(4)]
                for p in range(4):
                    nc.sync.dma_start(out=wtp[p], in_=wproj[p])

                for qc in range(2):
                    for h in range(H):
                        hp = h // 2
                        p0 = (h % 2) * 64
                        avp = ps_av()
                        for gg in range(4):
                            scp = ps_sc4()
                            for bb in range(4):
                                nc.tensor.matmul(
                                    scp[:, bb, :],
                                    k_sb[p0:p0 + 64, hp, ts(4 * gg + bb, 128)],
                                    q_all[p0:p0 + 64, hp, ts(qc, 512)],
                                    start=True, stop=True)
                            et = expp.tile([128, 4, 512], F8, name="et",
                                           tag="et")
                            nc.scalar.activation(out=et, in_=scp,
                                                 func=AF.Exp,
                                                 scale=ESCALE)
                            if "d_avp" in aps and qc == 0 and h == 0 \
                                    and gg == 0:
                                nc.sync.dma_start(out=aps["d_et"], in_=et)
                            for j in range(2):
                                nc.tensor.matmul(
                                    avp,
                                    v_aug[:, 4 * gg + 2 * j:
                                          4 * gg + 2 * j + 2, h, :],
                                    et[:, 2 * j:2 * j + 2, :],
                                    start=(gg == 0 and j == 0),
                                    stop=(gg == 3 and j == 1),
                                    perf_mode=PM.DoubleRow,
                                    skip_group_check=True)
                        r_t = misc.tile([1, 512], F32, name="r_t",
                                        tag="r", bufs=2)
                        nc.vector.reciprocal_approx_fast(
                            out=r_t, in_=avp[0:1, :])
                        rb_t = misc.tile([64, 512], F32, name="rb_t",
                                         tag="rb", bufs=2)
                        nc.gpsimd.partition_broadcast(rb_t, r_t)
                        if "d_avp" in aps and qc == 0 and h == 0:
                            avc = misc.tile([128, 512], F32, name="avc",
                                            tag="avc", bufs=1)
                            nc.vector.tensor_copy(out=avc, in_=avp)
                            nc.sync.dma_start(out=aps["d_avp"], in_=avc)
                            nc.sync.dma_start(out=aps["d_rt"], in_=r_t)
                            nc.sync.dma_start(out=aps["d_rb"], in_=rb_t)
                        nc.vector.tensor_mul(
                            out=o_sb[p0:p0 + 64, hp, ts(qc, 512)],
                            in0=avp[64:128, :], in1=rb_t)
                        # interleave MLP1(qc0) into qc1's head loop
                        if qc == 1:
                            mlp1_unit(0, 2 * h, _rot2)
                            mlp1_unit(0, 2 * h + 1, _rot2)
                    # proj + residual -> x2 (SBUF bf16), LN2 stats fused
                    x2 = mlpp.tile([128, 8, 512], BF, name="x2",
                                   tag="x2", bufs=2)
                    x2_tiles[qc] = x2
                    st2 = ps_st()
                    for m in range(NCC):
                        pp = ps_av()
                        for p in range(4):
                            nc.tensor.matmul(
                                pp, wtp[p][:, :, ts(m, 128)],
                                o_sb[:, 2 * p:2 * p + 2, ts(qc, 512)],
                                start=(p == 0), stop=(p == 3),
                                perf_mode=PM.DoubleRow,
                                skip_group_check=True)
                        xres = stage.tile([128, 512], F32, name="xres",
                                          tag="xres", bufs=2)
                        nc.sync.dma_start(out=xres,
                                          in_=xT[m, :, ts(qc, 512)])
                        nc.vector.scalar_tensor_tensor(
                            out=x2[:, m, :], in0=pp, scalar=1.0 / 1024.0,
                            in1=xres, op0=MULT, op1=ADD)
                        if has_bp:
                            nc.vector.tensor_scalar_add(
                                out=x2[:, m, :], in0=x2[:, m, :],
                                scalar1=bp_sb[:, m:m + 1])
                        xq2 = stage.tile([128, 512], BF, name="xq2",
                                         tag="xq2", bufs=2)
                        nc.vector.tensor_mul(out=xq2, in0=x2[:, m, :],
                                             in1=x2[:, m, :])
                        nc.tensor.matmul(st2[:, 0, :], ones_sc, x2[:, m, :],
                                         start=(m == 0),
                                         stop=(m == NCC - 1),
                                         skip_group_check=True)
                        nc.tensor.matmul(st2[:, 1, :], ones_sc, xq2,
                                         start=(m == 0),
                                         stop=(m == NCC - 1),
                                         skip_group_check=True)
                    ln2_stats[qc] = stats_finish(st2, bf16_out=False)
                    ln2_apply(qc)

                if "d_ksb" in aps:
                    nc.sync.dma_start(out=aps["d_ksb"], in_=k_sb)
                    nc.sync.dma_start(out=aps["d_qall"], in_=q_all)
                    nc.sync.dma_start(out=aps["d_vaug"], in_=v_aug)
                    nc.sync.dma_start(out=aps["d_osb"], in_=o_sb)

        # ============ scope B: MLP2(qc0), MLP1+MLP2 for qc1 ========
        def acc8():
            """All 8 psum banks as [128,512] accumulator views."""
            a, st = ps_sc4(), ps_st()
            return ([a[:, bb, :] for bb in range(4)]
                    + [st[:, j, :] for j in range(2)]
                    + [ps_av(), ps_av()])

        with tc.tile_pool(name="w2p", bufs=8) as w2p, \
             tc.tile_pool(name="outp", bufs=3) as outp:

            def mlp2(qc):
                z_sb = z_tiles[qc]
                x2 = x2_tiles[qc]
                accs = acc8()
                for k in range(NFC):
                    w2t = w2p.tile([128, 1024], BF, name="w2t", tag="w2s")
                    nc.sync.dma_start(out=w2t, in_=w2[k])
                    for m in range(NCC):
                        nc.tensor.matmul(accs[m], w2t[:, ts(m, 128)],
                                         z_sb[:, k, :], start=(k == 0),
                                         stop=(k == NFC - 1),
                                         skip_group_check=True)
                for m in range(NCC):
                    ot = outp.tile([128, 512], F32, name="ot", tag="ot")
                    nc.vector.scalar_tensor_tensor(
                        out=ot, in0=accs[m], scalar=b2_sb[:, m:m + 1],
                        in1=x2[:, m, :], op0=ADD, op1=ADD)
                    nc.sync.dma_start(out=outT[m, :, ts(qc, 512)], in_=ot)

            mlp2(0)
            _rotB = rot6_gen()
            for u in range(32):
                mlp1_unit(1, u, _rotB)
            mlp2(1)

            if "d_ksb" in aps:
                nc.sync.dma_start(out=aps["d_x2a"], in_=x2_tiles[0])
                nc.sync.dma_start(out=aps["d_x2b"], in_=x2_tiles[1])
                nc.sync.dma_start(out=aps["d_z0"], in_=z_tiles[0])
                nc.sync.dma_start(out=aps["d_h20"], in_=h2_tiles[0])


def _build(has_bv, has_bp):
    from concourse import bacc, mybir, tile
    F32 = mybir.dt.float32
    BF = mybir.dt.bfloat16
    F8 = mybir.dt.float8e4

    nc = bacc.Bacc("TRN2", target_bir_lowering=False, debug=False,
                   enable_asserts=False, num_devices=8)
    aps = {}
    aps["xT"] = nc.dram_tensor("xT", [8, 128, T], F32, kind="ExternalInput").ap()
    for n in ("wq", "wk", "wv", "wproj"):
        aps[n] = nc.dram_tensor(n, [4, 128, 2, 1024], F8,
                                kind="ExternalInput").ap()
    aps["w1"] = nc.dram_tensor("w1", [8, 128, 8, 512], BF,
                               kind="ExternalInput").ap()
    aps["w2"] = nc.dram_tensor("w2", [32, 128, 1024], BF,
                               kind="ExternalInput").ap()
    for n in ("bq", "bk", "bp", "b2c"):
        aps[n] = nc.dram_tensor(n, [128, 8], F32, kind="ExternalInput").ap()
    aps["b1c"] = nc.dram_tensor("b1c", [128, 32], F32, kind="ExternalInput").ap()
    if has_bv:
        aps["bvrow"] = nc.dram_tensor("bvrow", [1, 1024], F32,
                                      kind="ExternalInput").ap()
    aps["outT"] = nc.dram_tensor("outT", [8, 128, TO], F32,
                                 kind="ExternalOutput").ap()
    if _DEBUG:
        aps["d_ksb"] = nc.dram_tensor("d_ksb", [128, 8, T], BF,
                                      kind="ExternalOutput").ap()
        aps["d_qall"] = nc.dram_tensor("d_qall", [128, 8, TO], BF,
                                       kind="ExternalOutput").ap()
        aps["d_vaug"] = nc.dram_tensor("d_vaug", [128, 16, 16, 128], F8,
                                       kind="ExternalOutput").ap()
        aps["d_osb"] = nc.dram_tensor("d_osb", [128, 8, TO], F8,
                                      kind="ExternalOutput").ap()
        aps["d_x2a"] = nc.dram_tensor("d_x2a", [128, 8, 512], BF,
                                      kind="ExternalOutput").ap()
        aps["d_x2b"] = nc.dram_tensor("d_x2b", [128, 8, 512], BF,
                                      kind="ExternalOutput").ap()
        aps["d_z0"] = nc.dram_tensor("d_z0", [128, 32, 512], BF,
                                     kind="ExternalOutput").ap()
        aps["d_h20"] = nc.dram_tensor("d_h20", [128, 8, 512], BF,
                                      kind="ExternalOutput").ap()
        aps["d_avp"] = nc.dram_tensor("d_avp", [128, 512], F32,
                                      kind="ExternalOutput").ap()
        aps["d_rt"] = nc.dram_tensor("d_rt", [1, 512], F32,
                                     kind="ExternalOutput").ap()
        aps["d_rb"] = nc.dram_tensor("d_rb", [64, 512], F32,
                                     kind="ExternalOutput").ap()
        aps["d_et"] = nc.dram_tensor("d_et", [128, 4, 512], F8,
                                     kind="ExternalOutput").ap()

    with tile.TileContext(nc) as tcx:
        _emit(nc, tcx, aps, has_bv, has_bp)
    nc.compile()
    return nc


def _prep_inputs(x, Wq, Wk, Wv, Wproj, bproj, W1, b1, W2, b2, g1, be1, g2, be2):
    """Host-side prep: fold LN affine into weights/biases, cast, lay out."""
    x = np.asarray(x, np.float32)
    g1 = np.asarray(g1, np.float32)
    be1 = np.asarray(be1, np.float32)
    g2 = np.asarray(g2, np.float32)
    be2 = np.asarray(be2, np.float32)

    def to2d(w):  # (H, C, hd) -> (C, H*hd)
        return np.asarray(w, np.float32).transpose(1, 0, 2).reshape(C, C)

    wq2, wk2, wv2 = to2d(Wq), to2d(Wk), to2d(Wv)
    Wproj = np.asarray(Wproj, np.float32)
    W1 = np.asarray(W1, np.float32)
    W2 = np.asarray(W2, np.float32)

    wq_e, wk_e, wv_e = g1[:, None] * wq2, g1[:, None] * wk2, g1[:, None] * wv2
    w1_e = g2[:, None] * W1
    bias_q = 32.0 * (be1 @ wq2)   # k', q' carry x32
    bias_k = 32.0 * (be1 @ wk2)
    bias_v = be1 @ wv2            # v is descaled in its epilogue
    bias_1 = np.asarray(b1, np.float32) + be2 @ W1

    def wpair8(w):  # (C, N) -> (4, 128, 2, N) fp8, x32
        a = (32.0 * w).reshape(4, 2, 128, -1).transpose(0, 2, 1, 3)
        return np.ascontiguousarray(a.astype(F8NP))

    def bvec(v):  # (N,) -> (128, N//128) partition-major
        return np.ascontiguousarray(np.asarray(v, np.float32).reshape(-1, 128).T)

    shared = {
        "wq": wpair8(wq_e), "wk": wpair8(wk_e), "wv": wpair8(wv_e),
        "wproj": wpair8(Wproj),
        "w1": np.ascontiguousarray(
            w1_e.reshape(NCC, 128, 8, 512).transpose(2, 1, 0, 3).astype(BF16)),
        "w2": np.ascontiguousarray(W2.reshape(NFC, 128, C).astype(BF16)),
        "bq": bvec(bias_q), "bk": bvec(bias_k),
        "bp": bvec(np.asarray(bproj, np.float32)),
        "b2c": bvec(np.asarray(b2, np.float32)), "b1c": bvec(bias_1),
    }
    has_bv = bool(np.any(bias_v != 0.0))
    has_bp = bool(np.any(np.asarray(bproj, np.float32) != 0.0))
    if has_bv:
        shared["bvrow"] = np.ascontiguousarray(bias_v.reshape(1, C))

    in_maps = []
    for core in range(8):
        b, half = core // 2, core % 2
        xt = x[b].T  # (C, T)
        own = xt[:, half * TO:(half + 1) * TO]
        oth = xt[:, (1 - half) * TO:(2 - half) * TO]
        m = dict(shared)
        m["xT"] = np.ascontiguousarray(
            np.concatenate([own, oth], axis=1).reshape(NCC, 128, T))
        in_maps.append(m)
    return in_maps, has_bv, has_bp


def kernel(x, Wq, Wk, Wv, Wproj, bproj, W1, b1, W2, b2, g1, be1, g2, be2):
    global _BUILT
    from concourse.bass_utils import run_bass_kernel_spmd

    in_maps, has_bv, has_bp = _prep_inputs(x, Wq, Wk, Wv, Wproj, bproj, W1,
                                           b1, W2, b2, g1, be1, g2, be2)
    if _BUILT is None or _BUILT[1] != (has_bv, has_bp):
        _BUILT = (_build(has_bv, has_bp), (has_bv, has_bp))
    nc = _BUILT[0]
    res = run_bass_kernel_spmd(nc, in_maps, core_ids=list(range(8)))
    out = np.empty((B, T, C), np.float32)
    for core in range(8):
        b, half = core // 2, core % 2
        o = res.results[core]["outT"].reshape(C, TO)  # (feature, token)
        out[b, half * TO:(half + 1) * TO, :] = o.T
    return out


# revision 23
# speedup vs baseline: 1.2946x; 1.2946x over previous
"""Trainium2 Bass kernel for a dense transformer block (B=4, T=2048, C=1024, H=16).

Sharding: data-parallel over tokens. Core i owns batch b=i//2, token-half i%2
(1024 tokens). Each core redundantly computes LN1/K/V for its batch's full 2048
tokens so there are no collectives.

v3: restructured from the v2 pipeline for engine balance:
- Phase A (LN1+K/V/Q): x is read from HBM once; f32->bf16 casts moved to the
  (otherwise idle) gpsimd engine; LN applies run on the kept bf16 copy in 2x
  DVE mode (mu/s cast to bf16); Q is emitted inside its own chunk so it
  pipelines with the LN chain.
- Attention: scores land in a 4-bank PSUM tile so exp runs at N=2048 per
  ACTIVATE (amortizes the 352-cycle ACT overhead). Scores matmuls contract
  K=64 directly (single q_all tile, no zero-padded half-tiles, half the Q
  epilogues, no qz memsets). Softmax normalize uses reciprocal_approx_fast
  (~5x cheaper than the iterative divide; denominators are >=0.1 so the
  approx is safe).
- MLP1(qc0) interleaves into qc1's head loop at m4 granularity (8-matmul
  units, ~1.7us) so the exp stream is never starved by long PE detours.
- relu on DVE everywhere -> ACT stays on the Ln/Exp table set (no reloads).
- x2 (= x + sa) is kept in SBUF as bf16 (no DRAM round-trip for LN2/MLP2
  residuals; the ~0.4% bf16 residual quantization is well inside the 2e-2
  gate).
- v_aug: the 1/32 denominator in column 0 (the custom-DVE reciprocal
  only reads partition 0) and v feats in columns 64-127 (>32-partition
  PSUM accesses must start at partition 0 or 64).

Scaling (unchanged from v2): fp8 weights stored x32; k/q carry x32 (folded
into the exp scale C^-0.5/1024); v descaled by 1/32 in its epilogue; the
attention denominator rides AV as a (1/32)-column of V so o comes out x32,
matching the x32 proj weights; the proj epilogue multiplies by 2^-10.

PSUM: base pool avp [128,512] x2 + st [128,2,512] (4 banks, whole kernel)
plus a scoped 4-bank pool per phase: psA (phase-A K/V/Q accums), psB
(2x double-buffered f32 score tiles [128,2,512]), psC (MLP2 accums /
MLP1(1) rotation).
"""

import sys

if "/opt/trn_rl_repo" not in sys.path:
    sys.path.insert(0, "/opt/trn_rl_repo")

import numpy as np
import ml_dtypes

B, T, C, H, HD = 4, 2048, 1024, 16, 64
FF = 4 * C
TO = T // 2          # tokens owned per core
NCC = C // 128       # 8
NFC = FF // 128      # 32
EPS = 1e-5
SCALE = C ** -0.5    # 1/32
ESCALE = SCALE / 1024.0   # exp scale with k,q carried x32
BF16 = ml_dtypes.bfloat16
F8NP = ml_dtypes.float8_e4m3

_BUILT = None
_DEBUG = False   # when True, _build adds intermediate-dump outputs


def _emit(nc, tc, aps, has_bv, has_bp):
    from concourse import mybir
    from concourse.bass import ts
    F32 = mybir.dt.float32
    BF = mybir.dt.bfloat16
    F8 = mybir.dt.float8e4
    AF = mybir.ActivationFunctionType
    PM = mybir.MatmulPerfMode
    ADD = mybir.AluOpType.add
    MULT = mybir.AluOpType.mult
    MAX = mybir.AluOpType.max
    from contextlib import ExitStack

    xT, wq, wk, wv, wproj, w1, w2, outT = (
        aps["xT"], aps["wq"], aps["wk"], aps["wv"], aps["wproj"], aps["w1"],
        aps["w2"], aps["outT"])

    ctx = ExitStack()
    with ctx:
        const = ctx.enter_context(tc.tile_pool(name="const", bufs=1))
        misc = ctx.enter_context(tc.tile_pool(name="misc", bufs=2))
        psum = ctx.enter_context(tc.tile_pool(name="psum", bufs=1, space="PSUM"))

        def ps_st():
            return psum.tile([128, 2, 512], F32, name="ps_st", tag="st", bufs=1)

        def ps_av():
            return psum.tile([128, 512], F32, name="ps_av", tag="avp", bufs=2)

        def rot6_gen(p4):
            """6 rotating [128,512] accumulators (a scoped 4-bank f32 tile's
            views + 2 avp)."""
            while True:
                t = p4()
                for bb in range(4):
                    yield t[:, bb, :]
                yield ps_av()
                yield ps_av()

        def rot2_gen():
            """2 rotating accumulators clear of sc4/avp (for MLP1 interleaved
            into the attention head loop)."""
            while True:
                t = ps_st()
                yield t[:, 0, :]
                yield t[:, 1, :]

        # fresh generator per phase: a generator suspended mid-cycle holds a
        # live psum tile; resuming it in a later phase would make that stale
        # tile's release depend on later-phase work (deadlock)
        _rot2 = rot2_gen()

        # constants; ones_sc/eps_sb are built via gpsimd so its ucode IRAM
        # loads overlap the initial DMAs instead of stalling the first real
        # cast/broadcast (~6us each)
        ones_f = const.tile([128, 128], F32, name="ones_f")
        nc.vector.memset(ones_f, 1.0 / C)
        ones_sc = const.tile([128, 128], BF, name="ones_sc")
        nc.gpsimd.tensor_copy(out=ones_sc, in_=ones_f)
        eps1 = const.tile([1, 1], F32, name="eps1")
        nc.vector.memset(eps1, EPS)
        eps_sb = const.tile([128, 1], F32, name="eps_sb")
        nc.gpsimd.partition_broadcast(eps_sb, eps1)

        bq_sb = const.tile([128, 8], F32, name="bq_sb")
        bk_sb = const.tile([128, 8], F32, name="bk_sb")
        bp_sb = const.tile([128, 8], F32, name="bp_sb")
        b2_sb = const.tile([128, 8], F32, name="b2_sb")
        b1_sb = const.tile([128, 32], F32, name="b1_sb")
        nc.sync.dma_start(out=bq_sb, in_=aps["bq"])
        nc.sync.dma_start(out=bk_sb, in_=aps["bk"])
        nc.sync.dma_start(out=bp_sb, in_=aps["bp"])
        nc.sync.dma_start(out=b2_sb, in_=aps["b2c"])
        nc.sync.dma_start(out=b1_sb, in_=aps["b1c"])
        if has_bv:
            bv_sb = const.tile([1, 1024], F32, name="bv_sb")
            nc.sync.dma_start(out=bv_sb, in_=aps["bvrow"])
            bvb = const.tile([128, 1024], F32, name="bvb")
            nc.gpsimd.partition_broadcast(bvb, bv_sb)

        def stats_finish(st, bf16_out):
            """st: psum [:,0]=mean, [:,1]=E[x^2] (rows replicated).
            1/sd via exp(-0.5*ln(var+eps)) so ACT stays on the exp table."""
            mu0 = misc.tile([128, 512], BF if bf16_out else F32,
                            name="mu0", tag="mstat", bufs=4)
            nc.vector.tensor_copy(out=mu0, in_=st[:, 0, :])
            musq = misc.tile([128, 512], F32, name="musq", tag="stat", bufs=2)
            nc.vector.tensor_mul(out=musq, in0=mu0, in1=mu0)
            var = misc.tile([128, 512], F32, name="var", tag="stat", bufs=2)
            nc.vector.tensor_sub(out=var, in0=st[:, 1, :], in1=musq)
            lnv = misc.tile([128, 512], F32, name="lnv", tag="stat", bufs=2)
            nc.scalar.activation(out=lnv, in_=var, func=AF.Ln, bias=eps_sb)
            s0 = misc.tile([128, 512], F32, name="s0", tag="mstat", bufs=4)
            nc.scalar.activation(out=s0, in_=lnv, func=AF.Exp, scale=-0.5)
            if bf16_out:
                sb = misc.tile([128, 512], BF, name="sbf", tag="msbf", bufs=2)
                nc.gpsimd.tensor_copy(out=sb, in_=s0)
                return mu0, sb
            return mu0, s0

        ln2_stats = {}
        h2_tiles = {}
        z_tiles = {}
        x2_tiles = {}
        _w1_tiles = {}

        with tc.tile_pool(name="attn", bufs=1) as attn:
            k_sb = attn.tile([128, 8, T], BF, name="k_sb")
            q_all = attn.tile([128, 8, TO], BF, name="q_all")
            o_sb = attn.tile([128, 8, TO], F8, name="o_sb")
            v_aug = attn.tile([128, 16, 16, 128], F8, name="v_aug")
            # denominator col 0 = 1/32 (partition 0 of the AV psum: the
            # custom-DVE reciprocal reads its input at partition offset 0
            # regardless of the AP, so the denominator must live there)
            nc.vector.memset(v_aug[:, :, :, 0:1], 1.0 / 32.0)

            # ---- phase A: LN1 chunks + K/V (+Q for own chunks) ----
            with tc.tile_pool(name="lnp", bufs=2) as lnp, \
                 tc.tile_pool(name="wkv", bufs=4) as wkv, \
                 tc.tile_pool(name="psA", bufs=1, space="PSUM") as psA:
                def pA():
                    return psA.tile([128, 4, 512], F32, name="pA", tag="pA",
                                    bufs=1)
                _rot6 = rot6_gen(pA)
                wtk = wtv = wtq = None
                for tcg in range(4):
                    xbf = lnp.tile([128, 8, 512], BF, name="xbf", tag="xbf",
                                   bufs=2)
                    st = ps_st()
                    for c in range(NCC):
                        xs = lnp.tile([128, 512], F32, name="xs", tag="xs",
                                      bufs=4)
                        nc.sync.dma_start(out=xs, in_=xT[c, :, ts(tcg, 512)])
                        nc.vector.tensor_copy(out=xbf[:, c, :], in_=xs)
                        xsq = lnp.tile([128, 512], BF, name="xsq", tag="xsq",
                                       bufs=3)
                        nc.vector.tensor_mul(out=xsq, in0=xbf[:, c, :],
                                             in1=xbf[:, c, :])
                        nc.tensor.matmul(st[:, 0, :], ones_sc, xbf[:, c, :],
                                         start=(c == 0), stop=(c == NCC - 1),
                                         skip_group_check=True)
                        nc.tensor.matmul(st[:, 1, :], ones_sc, xsq,
                                         start=(c == 0), stop=(c == NCC - 1),
                                         skip_group_check=True)
                    if tcg == 0:
                        # weights emitted after chunk-0 x loads so x wins the
                        # DMA queue race and the LN pipeline starts early
                        wtk = [wkv.tile([128, 2, 1024], F8, name="wtk",
                                        tag="wkt") for _ in range(4)]
                        wtv = [wkv.tile([128, 2, 1024], F8, name="wtv",
                                        tag="wvt") for _ in range(4)]
                        wtq = [wkv.tile([128, 2, 1024], F8, name="wtq",
                                        tag="wqt") for _ in range(4)]
                        for p in range(4):
                            nc.sync.dma_start(out=wtk[p], in_=wk[p])
                            nc.sync.dma_start(out=wtv[p], in_=wv[p])
                        for p in range(4):
                            nc.sync.dma_start(out=wtq[p], in_=wq[p])
                    mu_bf, s_bf = stats_finish(st, bf16_out=True)
                    h_t = lnp.tile([128, 8, 512], F8, name="h_t", tag="h",
                                   bufs=2)
                    for c in range(NCC):
                        d = lnp.tile([128, 512], BF, name="d", tag="dt",
                                     bufs=3)
                        nc.vector.tensor_sub(out=d, in0=xbf[:, c, :],
                                             in1=mu_bf)
                        nc.vector.tensor_mul(out=h_t[:, c, :], in0=d,
                                             in1=s_bf)
                    # K: out k' = 32k [128 feats, 512 tok]
                    for m in range(NCC):
                        kp = next(_rot6)
                        for p in range(4):
                            nc.tensor.matmul(kp, wtk[p][:, :, ts(m, 128)],
                                             h_t[:, 2 * p:2 * p + 2, :],
                                             start=(p == 0), stop=(p == 3),
                                             perf_mode=PM.DoubleRow,
                                             skip_group_check=True)
                        nc.scalar.activation(
                            out=k_sb[:, m, ts(tcg, 512)], in_=kp,
                            func=AF.Identity, bias=bk_sb[:, m:m + 1])
                    # V: out [128 tok, 512 feats]; epilogue x(1/32) -> fp8
                    for tt in range(4):
                        for nch in range(2):
                            vp = next(_rot6)
                            for p in range(4):
                                nc.tensor.matmul(
                                    vp, h_t[:, 2 * p:2 * p + 2,
                                            ts(tt, 128)],
                                    wtv[p][:, :, ts(nch, 512)],
                                    start=(p == 0), stop=(p == 3),
                                    perf_mode=PM.DoubleRow,
                                    skip_group_check=True)
                            dst = v_aug[:, tcg * 4 + tt,
                                        nch * 8:(nch + 1) * 8, 64:128]
                            src = vp.rearrange("p (h d) -> p h d", h=8)
                            if has_bv:
                                vt = misc.tile([128, 8, 64], F32,
                                               name="vt", tag="vt", bufs=2)
                                bslice = bvb[:, ts(nch, 512)].rearrange(
                                    "p (h d) -> p h d", h=8)
                                nc.vector.tensor_add(out=vt, in0=src,
                                                     in1=bslice)
                                nc.scalar.mul(dst, vt, 1.0 / 32.0)
                            else:
                                nc.scalar.mul(dst, src, 1.0 / 32.0)
                    # Q projection for own chunks: q' = 32q, both head
                    # halves in one epilogue (full 128 partitions)
                    if tcg < 2:
                        for m in range(NCC):
                            qp = next(_rot6)
                            for p in range(4):
                                nc.tensor.matmul(
                                    qp, wtq[p][:, :, ts(m, 128)],
                                    h_t[:, 2 * p:2 * p + 2, :],
                                    start=(p == 0), stop=(p == 3),
                                    perf_mode=PM.DoubleRow,
                                    skip_group_check=True)
                            nc.scalar.activation(
                                out=q_all[:, m, ts(tcg, 512)], in_=qp,
                                func=AF.Identity, bias=bq_sb[:, m:m + 1])

            # ---- attention + proj per query chunk; MLP1(qc0) rides
            #      inside qc1's head loop at m4 granularity ----
            # side="right": these outlive the attn pool (non-LIFO vs the
            # left-side stack), so they allocate from the other heap end
            mlpp = ctx.enter_context(
                tc.tile_pool(name="mlp", bufs=1, side="right"))
            w1p = ctx.enter_context(
                tc.tile_pool(name="w1p", bufs=2, side="right"))

            def ln2_apply(qc):
                mu2, s2 = ln2_stats[qc]
                h2 = mlpp.tile([128, 8, 512], BF, name="h2", tag="h2", bufs=1)
                x2 = x2_tiles[qc]
                for c in range(NCC):
                    d2 = misc.tile([128, 512], F32, name="d2", tag="d2",
                                   bufs=2)
                    nc.vector.tensor_sub(out=d2, in0=x2[:, c, :], in1=mu2)
                    nc.vector.tensor_mul(out=h2[:, c, :], in0=d2, in1=s2)
                h2_tiles[qc] = h2
                z_tiles[qc] = mlpp.tile([128, 32, 512], BF, name="z_sb",
                                        tag="z", bufs=1)

            def mlp1_unit(qc, u, rot):
                """One m4 unit (8 matmuls + relu) of MLP1 for query chunk qc.
                u in 0..31; mg = u//4, m4 = u%4."""
                mg, m4 = u // 4, u % 4
                h2 = h2_tiles[qc]
                z_sb = z_tiles[qc]
                if m4 == 0:
                    w1t = w1p.tile([128, 8, 512], BF, name="w1t", tag="w1s")
                    nc.sync.dma_start(out=w1t, in_=w1[mg])
                    _w1_tiles[(qc, mg)] = w1t
                w1t = _w1_tiles[(qc, mg)]
                acc = next(rot)
                for k in range(NCC):
                    nc.tensor.matmul(acc, w1t[:, k, ts(m4, 128)],
                                     h2[:, k, :], start=(k == 0),
                                     stop=(k == NCC - 1),
                                     skip_group_check=True)
                m = mg * 4 + m4
                nc.vector.tensor_scalar(
                    out=z_sb[:, m, :], in0=acc,
                    scalar1=b1_sb[:, m:m + 1], scalar2=0.0,
                    op0=ADD, op1=MAX)

            with tc.tile_pool(name="wpp", bufs=4) as wpp, \
                 tc.tile_pool(name="expp", bufs=2) as expp, \
                 tc.tile_pool(name="stage", bufs=3) as stage, \
                 tc.tile_pool(name="psB", bufs=2, space="PSUM") as psB:
                def ps_scb():
                    return psB.tile([128, 2, 512], F32, name="scb",
                                    tag="scb", bufs=2)
                wtp = [wpp.tile([128, 2, 1024], F8, name="wtp", tag="wp")
                       for _ in range(4)]
                for p in range(4):
                    nc.sync.dma_start(out=wtp[p], in_=wproj[p])

                for qc in range(2):
                    for h in range(H):
                        hp = h // 2
                        p0 = (h % 2) * 64
                        avp = ps_av()
                        for gg in range(8):
                            scp = ps_scb()
                            for bb in range(2):
                                nc.tensor.matmul(
                                    scp[:, bb, :],
                                    k_sb[p0:p0 + 64, hp, ts(2 * gg + bb, 128)],
                                    q_all[p0:p0 + 64, hp, ts(qc, 512)],
                                    start=True, stop=True)
                            et = expp.tile([128, 2, 512], F8, name="et",
                                           tag="et")
                            nc.scalar.activation(out=et, in_=scp,
                                                 func=AF.Exp,
                                                 scale=ESCALE)
                            nc.tensor.matmul(
                                avp,
                                v_aug[:, 2 * gg:2 * gg + 2, h, :],
                                et,
                                start=(gg == 0), stop=(gg == 7),
                                perf_mode=PM.DoubleRow,
                                skip_group_check=True)
                        r_t = misc.tile([1, 512], F32, name="r_t",
                                        tag="r", bufs=2)
                        nc.vector.reciprocal_approx_fast(
                            out=r_t, in_=avp[0:1, :])
                        rb_t = misc.tile([64, 512], F32, name="rb_t",
                                         tag="rb", bufs=2)
                        nc.gpsimd.partition_broadcast(rb_t, r_t)
                        if "d_avp" in aps and qc == 0 and h == 0:
                            avc = misc.tile([128, 512], F32, name="avc",
                                            tag="avc", bufs=1)
                            nc.vector.tensor_copy(out=avc, in_=avp)
                            nc.sync.dma_start(out=aps["d_avp"], in_=avc)
                            nc.sync.dma_start(out=aps["d_rt"], in_=r_t)
                            nc.sync.dma_start(out=aps["d_rb"], in_=rb_t)
                        nc.vector.tensor_mul(
                            out=o_sb[p0:p0 + 64, hp, ts(qc, 512)],
                            in0=avp[64:128, :], in1=rb_t)
                        # interleave MLP1(qc0) into qc1's head loop
                        if qc == 1:
                            mlp1_unit(0, 2 * h, _rot2)
                            mlp1_unit(0, 2 * h + 1, _rot2)
                    # proj + residual -> x2 (SBUF bf16), LN2 stats fused
                    x2 = mlpp.tile([128, 8, 512], BF, name="x2",
                                   tag="x2", bufs=2)
                    x2_tiles[qc] = x2
                    st2 = ps_st()
                    for m in range(NCC):
                        pp = ps_av()
                        for p in range(4):
                            nc.tensor.matmul(
                                pp, wtp[p][:, :, ts(m, 128)],
                                o_sb[:, 2 * p:2 * p + 2, ts(qc, 512)],
                                start=(p == 0), stop=(p == 3),
                                perf_mode=PM.DoubleRow,
                                skip_group_check=True)
                        xres = stage.tile([128, 512], F32, name="xres",
                                          tag="xres", bufs=2)
                        nc.sync.dma_start(out=xres,
                                          in_=xT[m, :, ts(qc, 512)])
                        nc.vector.scalar_tensor_tensor(
                            out=x2[:, m, :], in0=pp, scalar=1.0 / 1024.0,
                            in1=xres, op0=MULT, op1=ADD)
                        if has_bp:
                            nc.vector.tensor_scalar_add(
                                out=x2[:, m, :], in0=x2[:, m, :],
                                scalar1=bp_sb[:, m:m + 1])
                        xq2 = stage.tile([128, 512], BF, name="xq2",
                                         tag="xq2", bufs=2)
                        nc.vector.tensor_mul(out=xq2, in0=x2[:, m, :],
                                             in1=x2[:, m, :])
                        nc.tensor.matmul(st2[:, 0, :], ones_sc, x2[:, m, :],
                                         start=(m == 0),
                                         stop=(m == NCC - 1),
                                         skip_group_check=True)
                        nc.tensor.matmul(st2[:, 1, :], ones_sc, xq2,
                                         start=(m == 0),
                                         stop=(m == NCC - 1),
                                         skip_group_check=True)
                    ln2_stats[qc] = stats_finish(st2, bf16_out=False)
                    ln2_apply(qc)

                if "d_ksb" in aps:
                    nc.sync.dma_start(out=aps["d_ksb"], in_=k_sb)
                    nc.sync.dma_start(out=aps["d_qall"], in_=q_all)
                    nc.sync.dma_start(out=aps["d_vaug"], in_=v_aug)
                    nc.sync.dma_start(out=aps["d_osb"], in_=o_sb)

        # ============ scope B: MLP2(qc0), MLP1+MLP2 for qc1 ========
        with tc.tile_pool(name="w2p", bufs=8) as w2p, \
             tc.tile_pool(name="outp", bufs=3) as outp, \
             tc.tile_pool(name="psC", bufs=1, space="PSUM") as psC:
            def pC():
                return psC.tile([128, 4, 512], F32, name="pC", tag="pC",
                                bufs=1)

            def acc8():
                """All 8 psum banks as [128,512] accumulator views."""
                a, st = pC(), ps_st()
                return ([a[:, bb, :] for bb in range(4)]
                        + [st[:, j, :] for j in range(2)]
                        + [ps_av(), ps_av()])

            def mlp2(qc):
                z_sb = z_tiles[qc]
                x2 = x2_tiles[qc]
                accs = acc8()
                for k in range(NFC):
                    w2t = w2p.tile([128, 1024], BF, name="w2t", tag="w2s")
                    nc.sync.dma_start(out=w2t, in_=w2[k])
                    for m in range(NCC):
                        nc.tensor.matmul(accs[m], w2t[:, ts(m, 128)],
                                         z_sb[:, k, :], start=(k == 0),
                                         stop=(k == NFC - 1),
                                         skip_group_check=True)
                for m in range(NCC):
                    ot = outp.tile([128, 512], F32, name="ot", tag="ot")
                    nc.vector.scalar_tensor_tensor(
                        out=ot, in0=accs[m], scalar=b2_sb[:, m:m + 1],
                        in1=x2[:, m, :], op0=ADD, op1=ADD)
                    nc.sync.dma_start(out=outT[m, :, ts(qc, 512)], in_=ot)

            mlp2(0)
            _rotB = rot6_gen(pC)
            for u in range(32):
                mlp1_unit(1, u, _rotB)
            mlp2(1)

            if "d_ksb" in aps:
                nc.sync.dma_start(out=aps["d_x2a"], in_=x2_tiles[0])
                nc.sync.dma_start(out=aps["d_x2b"], in_=x2_tiles[1])
                nc.sync.dma_start(out=aps["d_z0"], in_=z_tiles[0])
                nc.sync.dma_start(out=aps["d_h20"], in_=h2_tiles[0])


def _build(has_bv, has_bp):
    from concourse import bacc, mybir, tile
    F32 = mybir.dt.float32
    BF = mybir.dt.bfloat16
    F8 = mybir.dt.float8e4

    nc = bacc.Bacc("TRN2", target_bir_lowering=False, debug=False,
                   enable_asserts=False, num_devices=8)
    aps = {}
    aps["xT"] = nc.dram_tensor("xT", [8, 128, T], F32, kind="ExternalInput").ap()
    for n in ("wq", "wk", "wv", "wproj"):
        aps[n] = nc.dram_tensor(n, [4, 128, 2, 1024], F8,
                                kind="ExternalInput").ap()
    aps["w1"] = nc.dram_tensor("w1", [8, 128, 8, 512], BF,
                               kind="ExternalInput").ap()
    aps["w2"] = nc.dram_tensor("w2", [32, 128, 1024], BF,
                               kind="ExternalInput").ap()
    for n in ("bq", "bk", "bp", "b2c"):
        aps[n] = nc.dram_tensor(n, [128, 8], F32, kind="ExternalInput").ap()
    aps["b1c"] = nc.dram_tensor("b1c", [128, 32], F32, kind="ExternalInput").ap()
    if has_bv:
        aps["bvrow"] = nc.dram_tensor("bvrow", [1, 1024], F32,
                                      kind="ExternalInput").ap()
    aps["outT"] = nc.dram_tensor("outT", [8, 128, TO], F32,
                                 kind="ExternalOutput").ap()
    if _DEBUG:
        aps["d_ksb"] = nc.dram_tensor("d_ksb", [128, 8, T], BF,
                                      kind="ExternalOutput").ap()
        aps["d_qall"] = nc.dram_tensor("d_qall", [128, 8, TO], BF,
                                       kind="ExternalOutput").ap()
        aps["d_vaug"] = nc.dram_tensor("d_vaug", [128, 16, 16, 128], F8,
                                       kind="ExternalOutput").ap()
        aps["d_osb"] = nc.dram_tensor("d_osb", [128, 8, TO], F8,
                                      kind="ExternalOutput").ap()
        aps["d_x2a"] = nc.dram_tensor("d_x2a", [128, 8, 512], BF,
                                      kind="ExternalOutput").ap()
        aps["d_x2b"] = nc.dram_tensor("d_x2b", [128, 8, 512], BF,
                                      kind="ExternalOutput").ap()
        aps["d_z0"] = nc.dram_tensor("d_z0", [128, 32, 512], BF,
                                     kind="ExternalOutput").ap()
        aps["d_h20"] = nc.dram_tensor("d_h20", [128, 8, 512], BF,
                                      kind="ExternalOutput").ap()
        aps["d_avp"] = nc.dram_tensor("d_avp", [128, 512], F32,
                                      kind="ExternalOutput").ap()
        aps["d_rt"] = nc.dram_tensor("d_rt", [1, 512], F32,
                                     kind="ExternalOutput").ap()
        aps["d_rb"] = nc.dram_tensor("d_rb", [64, 512], F32,
                                     kind="ExternalOutput").ap()
        aps["d_et"] = nc.dram_tensor("d_et", [128, 2, 512], F8,
                                     kind="ExternalOutput").ap()

    with tile.TileContext(nc) as tcx:
        _emit(nc, tcx, aps, has_bv, has_bp)
    nc.compile()
    return nc


def _prep_inputs(x, Wq, Wk, Wv, Wproj, bproj, W1, b1, W2, b2, g1, be1, g2, be2):
    """Host-side prep: fold LN affine into weights/biases, cast, lay out."""
    x = np.asarray(x, np.float32)
    g1 = np.asarray(g1, np.float32)
    be1 = np.asarray(be1, np.float32)
    g2 = np.asarray(g2, np.float32)
    be2 = np.asarray(be2, np.float32)

    def to2d(w):  # (H, C, hd) -> (C, H*hd)
        return np.asarray(w, np.float32).transpose(1, 0, 2).reshape(C, C)

    wq2, wk2, wv2 = to2d(Wq), to2d(Wk), to2d(Wv)
    Wproj = np.asarray(Wproj, np.float32)
    W1 = np.asarray(W1, np.float32)
    W2 = np.asarray(W2, np.float32)

    wq_e, wk_e, wv_e = g1[:, None] * wq2, g1[:, None] * wk2, g1[:, None] * wv2
    w1_e = g2[:, None] * W1
    bias_q = 32.0 * (be1 @ wq2)   # k', q' carry x32
    bias_k = 32.0 * (be1 @ wk2)
    bias_v = be1 @ wv2            # v is descaled in its epilogue
    bias_1 = np.asarray(b1, np.float32) + be2 @ W1

    def wpair8(w):  # (C, N) -> (4, 128, 2, N) fp8, x32
        a = (32.0 * w).reshape(4, 2, 128, -1).transpose(0, 2, 1, 3)
        return np.ascontiguousarray(a.astype(F8NP))

    def bvec(v):  # (N,) -> (128, N//128) partition-major
        return np.ascontiguousarray(np.asarray(v, np.float32).reshape(-1, 128).T)

    shared = {
        "wq": wpair8(wq_e), "wk": wpair8(wk_e), "wv": wpair8(wv_e),
        "wproj": wpair8(Wproj),
        "w1": np.ascontiguousarray(
            w1_e.reshape(NCC, 128, 8, 512).transpose(2, 1, 0, 3).astype(BF16)),
        "w2": np.ascontiguousarray(W2.reshape(NFC, 128, C).astype(BF16)),
        "bq": bvec(bias_q), "bk": bvec(bias_k),
        "bp": bvec(np.asarray(bproj, np.float32)),
        "b2c": bvec(np.asarray(b2, np.float32)), "b1c": bvec(bias_1),
    }
    has_bv = bool(np.any(bias_v != 0.0))
    has_bp = bool(np.any(np.asarray(bproj, np.float32) != 0.0))
    if has_bv:
        shared["bvrow"] = np.ascontiguousarray(bias_v.reshape(1, C))

    in_maps = []
    for core in range(8):
        b, half = core // 2, core % 2
        xt = x[b].T  # (C, T)
        own = xt[:, half * TO:(half + 1) * TO]
        oth = xt[:, (1 - half) * TO:(2 - half) * TO]
        m = dict(shared)
        m["xT"] = np.ascontiguousarray(
            np.concatenate([own, oth], axis=1).reshape(NCC, 128, T))
        in_maps.append(m)
    return in_maps, has_bv, has_bp


def kernel(x, Wq, Wk, Wv, Wproj, bproj, W1, b1, W2, b2, g1, be1, g2, be2):
    global _BUILT
    from concourse.bass_utils import run_bass_kernel_spmd

    in_maps, has_bv, has_bp = _prep_inputs(x, Wq, Wk, Wv, Wproj, bproj, W1,
                                           b1, W2, b2, g1, be1, g2, be2)
    if _BUILT is None or _BUILT[1] != (has_bv, has_bp):
        _BUILT = (_build(has_bv, has_bp), (has_bv, has_bp))
    nc = _BUILT[0]
    res = run_bass_kernel_spmd(nc, in_maps, core_ids=list(range(8)))
    out = np.empty((B, T, C), np.float32)
    for core in range(8):
        b, half = core // 2, core % 2
        o = res.results[core]["outT"].reshape(C, TO)  # (feature, token)
        out[b, half * TO:(half + 1) * TO, :] = o.T
    return out
